# revision 14
# baseline (speedup 1.0000x reference)
"""Trainium2 Bass kernel for nn_DevignModel (heterogeneous GNN message passing).

Self-contained: host preprocessing (node relabel by type, 8-way dst sharding,
(etype, dst)-sorted edges packed into segment-size classes) + Bass/Tile SPMD
program for 8 NeuronCores + output assembly.

Per layer on device:
  stage A: per 128-node chunk: K/Q/V projections (PE), nq head-dot, bf16 node
           tables -> DRAM; AllGather of the [k|v] table across the 8 cores.
  edge pass: dma_gather(transpose) of [k|v] rows -> feature-major stationary
           operand; one matmul vs per-relation rhs [A*pri | afc2*a_src | M]
           -> edge-major K~/nk~/V~; attention dot + leaky-relu gate; exp;
           segment softmax + message aggregation via constant P_s matmuls;
           dma_scatter_add of per-segment messages into per-node accumulators
           (parity ping-pong tables to avoid RMW races).
  stage D: cross-etype mean, Wa skip-mix, LayerNorm (feature-major).
"""
import os
import numpy as np
import ml_dtypes

D_IN, D, H, DK, L_FULL, T, R = 128, 64, 4, 16, 4, 3, 32
SQRT_DK = 4.0
NC_CORES = 8
P = 128
SMAX = 16
GCALL_SUB = 7   # subtiles per gather call (896 idxs; transpose-gather HW ring limit)
SCALL_SUB = 7    # subtiles per scatter call (aligned to gather groups)
QBATCH = 4       # subtiles batched per PSUM group

SRC_NT = np.array([0 if e <= 9 else (1 if e <= 21 else 2) for e in range(R)], dtype=np.int64)
def _dst_nt(e):
    if e <= 2 or 10 <= e <= 13 or 22 <= e <= 24: return 0
    if 3 <= e <= 6 or 14 <= e <= 17 or 25 <= e <= 28: return 1
    return 2
DST_NT = np.array([_dst_nt(e) for e in range(R)], dtype=np.int64)


def _wrap16(ix):
    """dma_gather/scatter idx layout: element i at [i%16, i//16], replicated
    across the 8 q7 cores (128 partitions)."""
    ix = np.asarray(ix, np.int16)
    out = ix.reshape(len(ix) // 16, 16).T.copy()
    return np.tile(out, (8, 1))


def _bf16(x):
    return np.asarray(x, np.float32).astype(ml_dtypes.bfloat16)


# ----------------------------------------------------------------- host prep
def _prep(src, dst, etype, ntype):
    N = len(ntype)
    order = np.argsort(ntype, kind="stable")
    # deal each type's nodes round-robin across shards -> balanced type mix
    raw_shards = [[] for _ in range(NC_CORES)]
    for t in range(T):
        ids_t = order[np.asarray(ntype)[order] == t]
        for c in range(NC_CORES):
            raw_shards[c].append(ids_t[c::NC_CORES])
    raw_shards = [np.concatenate(s) for s in raw_shards]
    nch = np.zeros(T, np.int64)
    for c in range(NC_CORES):
        tys = ntype[raw_shards[c]]
        for t in range(T):
            nch[t] = max(nch[t], -(-int((tys == t).sum()) // P))
    nch[T - 1] += 1  # guaranteed dummy chunk per shard
    chunk_types = np.concatenate([np.full(nch[t], t, np.int64) for t in range(T)])
    dn = int(nch.sum()) * P
    N_pad = dn * NC_CORES

    new2old = np.full(N_pad, -1, np.int64)
    for c in range(NC_CORES):
        ids = raw_shards[c]; tys = ntype[ids]
        off = c * dn
        for t in range(T):
            sel = ids[tys == t]
            new2old[off:off + len(sel)] = sel
            off += int(nch[t]) * P
    old2new = np.full(N, -1, np.int64)
    real = new2old >= 0
    old2new[new2old[real]] = np.nonzero(real)[0]
    assert (old2new >= 0).all()

    src_n = old2new[np.asarray(src)]
    dst_n = old2new[np.asarray(dst)]
    et = np.asarray(etype, np.int64)
    core_of = dst_n // dn

    per_core = []
    for c in range(NC_CORES):
        em = np.nonzero(core_of == c)[0]
        dstl = dst_n[em] - c * dn
        bet = et[em]
        o = np.lexsort((dstl, bet))
        em, dstl, bet = em[o], dstl[o], bet[o]
        blocks = []
        for r in range(R):
            sel = np.nonzero(bet == r)[0]
            bd, be = dstl[sel], em[sel]
            if len(sel):
                cut = np.nonzero(np.diff(bd))[0] + 1
                starts = np.concatenate([[0], cut]); ends = np.concatenate([cut, [len(bd)]])
            else:
                starts = ends = np.array([], np.int64)
            assert (ends - starts).max(initial=0) <= SMAX
            cls = {}
            for s0, e0 in zip(starts, ends):
                cls.setdefault(int(e0 - s0), []).append((int(s0), int(e0)))
            blocks.append({"cls": cls, "dstl": bd})
        per_core.append(blocks)

    nsub = np.zeros((R, SMAX + 1), np.int64)
    for c in range(NC_CORES):
        for r in range(R):
            for s, lst in per_core[c][r]["cls"].items():
                nsub[r, s] = max(nsub[r, s], -(-len(lst) // (P // s)))

    cores = []
    for c in range(NC_CORES):
        dummy = dn - 1
        e_src, e_dstl, seg_dstl = [], [], []
        for r in range(R):
            b = per_core[c][r]
            for s in range(1, SMAX + 1):
                if nsub[r, s] == 0:
                    continue
                cap = P // s
                segs = b["cls"].get(s, [])
                for ti in range(int(nsub[r, s])):
                    for k in range(cap):
                        gi = ti * cap + k
                        if gi < len(segs):
                            s0, e0 = segs[gi]
                            dl = int(b["dstl"][s0])
                            e_dstl.extend([dl] * s)
                            seg_dstl.append(dl)
                        else:
                            e_dstl.extend([dummy] * s)
                            seg_dstl.append(dummy)
                    tail = P - cap * s
                    e_dstl.extend([dummy] * tail)
                    seg_dstl.extend([dummy] * (P - cap))
        cores.append({"e_dstl": np.array(e_dstl, np.int64),
                      "seg_dstl": np.array(seg_dstl, np.int64)})

    # second pass: e_src needs the per-seg edge id ranges (kept separately to
    # avoid storing eids in blocks twice)
    for c in range(NC_CORES):
        em = np.nonzero(core_of == c)[0]
        dstl = dst_n[em] - c * dn
        bet = et[em]
        o = np.lexsort((dstl, bet))
        em, dstl, bet = em[o], dstl[o], bet[o]
        dummy_g = c * dn + (dn - 1)
        e_src = []
        for r in range(R):
            sel = np.nonzero(bet == r)[0]
            bd, be = dstl[sel], em[sel]
            if len(sel):
                cut = np.nonzero(np.diff(bd))[0] + 1
                starts = np.concatenate([[0], cut]); ends = np.concatenate([cut, [len(bd)]])
            else:
                starts = ends = np.array([], np.int64)
            cls = {}
            for s0, e0 in zip(starts, ends):
                cls.setdefault(int(e0 - s0), []).append((int(s0), int(e0)))
            for s in range(1, SMAX + 1):
                if nsub[r, s] == 0:
                    continue
                cap = P // s
                segs = cls.get(s, [])
                for ti in range(int(nsub[r, s])):
                    for k in range(cap):
                        gi = ti * cap + k
                        if gi < len(segs):
                            s0, e0 = segs[gi]
                            e_src.extend(src_n[be[s0:e0]].tolist())
                        else:
                            e_src.extend([dummy_g] * s)
                    e_src.extend([dummy_g] * (P - cap * s))
        cores[c]["e_src"] = np.array(e_src, np.int64)
        assert len(cores[c]["e_src"]) == len(cores[c]["e_dstl"])

    sub_meta = []
    for r in range(R):
        for s in range(1, SMAX + 1):
            sub_meta += [(r, s)] * int(nsub[r, s])
    classes = sorted({s for (_, s) in sub_meta})

    npres = np.ones((NC_CORES, dn), np.float32)
    seg_all = np.unique(dst_n * R + et)
    cnt = np.zeros(N_pad, np.float32)
    np.add.at(cnt, seg_all // R, 1.0)
    for c in range(NC_CORES):
        npres[c] = np.maximum(cnt[c * dn:(c + 1) * dn], 1.0)

    return {"dn": dn, "N_pad": N_pad, "new2old": new2old, "old2new": old2new,
            "chunk_types": chunk_types, "cores": cores, "npres": npres,
            "nsub": nsub, "sub_meta": sub_meta, "classes": classes}


# ------------------------------------------------------------- program build
_PROG_CACHE = {}


def _build(pp, L, scal):
    DO_EDGE = os.environ.get("K_EDGE", "1") != "0"
    DO_GATHER = os.environ.get("K_GATHER", "1") != "0"
    DO_SCATTER = os.environ.get("K_SCATTER", "1") != "0"
    DO_CC = os.environ.get("K_CC", "1") != "0"

    import concourse.bass as bass
    import concourse.bacc as bacc
    import concourse.mybir as mybir
    import concourse.tile as tile
    from contextlib import ExitStack

    dt = mybir.dt
    AF = mybir.ActivationFunctionType
    ALU = mybir.AluOpType

    dn, N_pad = pp["dn"], pp["N_pad"]
    nchunk = dn // P
    chunk_types = pp["chunk_types"]
    sub_meta = pp["sub_meta"]
    classes = pp["classes"]
    nsubtot = len(sub_meta)
    nslots = nsubtot * P

    nc = bacc.Bacc("TRN2", target_bir_lowering=False, debug=False,
                   num_devices=NC_CORES, dynamic_dma_scratch_size=16384)

    p_h = nc.dram_tensor("h_shard", [dn, D_IN], dt.float32, kind="ExternalInput")
    p_adW = nc.dram_tensor("adW", [T, D_IN, D], dt.float32, kind="ExternalInput")
    p_adB = nc.dram_tensor("adB", [T, D, 1], dt.float32, kind="ExternalInput")
    p_pW = nc.dram_tensor("pW", [L, T, D, 3 * D], dt.float32, kind="ExternalInput")
    p_pB = nc.dram_tensor("pB", [L, T, P, 3 * D], dt.float32, kind="ExternalInput")
    p_WaT = nc.dram_tensor("WaT", [L, T, D, D], dt.float32, kind="ExternalInput")
    p_baC = nc.dram_tensor("baC", [L, T, D, 1], dt.float32, kind="ExternalInput")
    p_lnG = nc.dram_tensor("lnG", [L, T, D, 1], dt.float32, kind="ExternalInput")
    p_lnB = nc.dram_tensor("lnB", [L, T, D, 1], dt.float32, kind="ExternalInput")
    p_TR = nc.dram_tensor("TRr", [L, R, P, 132], dt.bfloat16, kind="ExternalInput")
    p_Ps = nc.dram_tensor("Ps", [len(classes), P, P], dt.bfloat16, kind="ExternalInput")
    p_afc1 = nc.dram_tensor("afc1rep", [L, P, D], dt.float32, kind="ExternalInput")
    p_ident = nc.dram_tensor("ident", [P, P], dt.float32, kind="ExternalInput")
    p_npi = nc.dram_tensor("npres_inv", [P, nchunk], dt.float32, kind="ExternalInput")
    p_kvix = nc.dram_tensor("kv_idx", [P, nslots // 16], dt.int16, kind="ExternalInput")
    p_qix = nc.dram_tensor("q_idx", [P, nslots // 16], dt.int16, kind="ExternalInput")
    p_scix = nc.dram_tensor("sc_idx", [P, nslots // 16], dt.int16, kind="ExternalInput")

    o_x = nc.dram_tensor("x_out", [dn, D], dt.float32, kind="ExternalOutput")

    # kv stripes layout: [128 parts, nchunk, 128 feats]; node ch*128+p at [p, ch, :]
    kv_shard = nc.dram_tensor("kv_shard", [P, nchunk * 128], dt.bfloat16)
    kv_full = nc.dram_tensor("kv_full", [NC_CORES, P, nchunk * 128], dt.bfloat16,
                             addr_space="Shared")
    q_loc = nc.dram_tensor("q_loc", [dn, 128], dt.bfloat16)
    nacc = -(-nchunk // 2)  # accumulator free-dim blocks per parity

    def bch(apx, n=DK):
        """[p, H] -> [p, H, n] broadcast (step-0 inner dim)."""
        return apx.to_broadcast([apx.shape[0], H, n])

    with tile.TileContext(nc) as tc, ExitStack() as ST:
        cpool = ST.enter_context(tc.tile_pool(name="consts", bufs=1))
        ident = cpool.tile([P, P], dt.float32)
        nc.sync.dma_start(ident[:], p_ident[:])
        adW, adB = {}, {}
        for t in range(T):
            adW[t] = cpool.tile([D_IN, D], dt.float32, tag=f"adW{t}", name=f"adW{t}")
            nc.sync.dma_start(adW[t][:], p_adW[t])
            adB[t] = cpool.tile([D, 1], dt.float32, tag=f"adB{t}", name=f"adB{t}")
            nc.sync.dma_start(adB[t][:], p_adB[t])
        pW, pB, WaT, baC, lnG, lnB = {}, {}, {}, {}, {}, {}
        for l in range(L):
            for t in range(T):
                for nm, store, par, shp in (
                        ("pW", pW, p_pW, [D, 3 * D]), ("pB", pB, p_pB, [P, 3 * D]),
                        ("Wa", WaT, p_WaT, [D, D]), ("ba", baC, p_baC, [D, 1]),
                        ("lg", lnG, p_lnG, [D, 1]), ("lb", lnB, p_lnB, [D, 1])):
                    store[l, t] = cpool.tile(shp, dt.float32, tag=f"{nm}{l}_{t}", name=f"{nm}{l}_{t}")
                    nc.sync.dma_start(store[l, t][:], par[l, t])
        TRt = {}
        for l in range(L):
            for r in range(R):
                TRt[l, r] = cpool.tile([P, 132], dt.bfloat16, tag=f"TR{l}_{r}", name=f"TR{l}_{r}")
                nc.sync.dma_start(TRt[l, r][:], p_TR[l, r])
        Pst = {}
        for ci, s in enumerate(classes):
            Pst[s] = cpool.tile([P, P], dt.bfloat16, tag=f"Ps{s}", name=f"Ps{s}")
            nc.sync.dma_start(Pst[s][:], p_Ps[ci])
        afc1 = {}
        for l in range(L):
            afc1[l] = cpool.tile([P, D], dt.float32, tag=f"afc1_{l}", name=f"afc1_{l}")
            nc.sync.dma_start(afc1[l][:], p_afc1[l])
        npi = cpool.tile([P, nchunk], dt.float32)
        nc.sync.dma_start(npi[:], p_npi[:])
        kvix = cpool.tile([P, nslots // 16], dt.int16, tag="kvix", name="kvix")
        nc.sync.dma_start(kvix[:], p_kvix[:])
        qix = cpool.tile([P, nslots // 16], dt.int16, tag="qix", name="qix")
        nc.sync.dma_start(qix[:], p_qix[:])
        scix = cpool.tile([P, nslots // 16], dt.int16, tag="scix", name="scix")
        nc.sync.dma_start(scix[:], p_scix[:])
        onesMu = cpool.tile([D, 1], dt.float32, tag="onesMu", name="onesMu")
        nc.vector.memset(onesMu[:], 1.0 / D)
        onesB = cpool.tile([1, D], dt.float32, tag="onesB", name="onesB")
        nc.vector.memset(onesB[:], 1.0)
        zero64 = cpool.tile([P, D], dt.float32, tag="zero64", name="zero64")
        nc.vector.memset(zero64[:], 0.0)
        c_zero = cpool.tile([P, 1], dt.float32, tag="c_zero", name="c_zero")
        nc.vector.memset(c_zero[:], 0.0)
        c_eps = cpool.tile([1, 1], dt.float32, tag="c_eps", name="c_eps")
        nc.vector.memset(c_eps[:], 1e-5)
        xT = [cpool.tile([D, dn], dt.float32, tag=f"xT{i}", name=f"xT{i}") for i in range(2)]
        # SBUF kv table (gather source) + SBUF scatter accumulators
        kvtab = cpool.tile([P, NC_CORES * nchunk * 128], dt.bfloat16, tag="kvtab", name="kvtab")
        acc = {}
        for pp_ in range(2):          # relation ping-pong
            for par in range(2):      # stripe parity (even/odd chunk)
                acc[pp_, par] = cpool.tile([P, nacc * D], dt.float32,
                                           tag=f"acc{pp_}_{par}", name=f"acc{pp_}_{par}")

        # -------- stage 0: adapt
        with tc.tile_pool(name="s0", bufs=3) as s0p, \
             tc.tile_pool(name="s0ps", bufs=2, space="PSUM") as s0ps:
            for ch in range(nchunk):
                t = int(chunk_types[ch])
                hch = s0p.tile([P, D_IN], dt.float32, tag="hch", name="hch")
                nc.sync.dma_start(hch[:], p_h[ch * P:(ch + 1) * P, :])
                hT_ps = s0ps.tile([P, P], dt.float32, tag="hT", name="hT")
                nc.tensor.transpose(hT_ps[:], hch[:], ident[:])
                hT = s0p.tile([D_IN, P], dt.float32, tag="hTsb", name="hTsb")
                nc.vector.tensor_copy(hT[:], hT_ps[:])
                x0 = s0ps.tile([D, P], dt.float32, tag="x0", name="x0")
                nc.tensor.matmul(x0[:], adW[t][:], hT[:], start=True, stop=True)
                nc.scalar.activation(xT[0][:, ch * P:(ch + 1) * P], x0[:],
                                     AF.Tanh, bias=adB[t][:, 0:1], scale=1.0)

        blk_start = {}
        pos = 0
        for r in range(R):
            blk_start[r] = pos
            pos += sum(int(pp["nsub"][r, s]) for s in range(1, SMAX + 1))

        for l in range(L):
            xin, xout = xT[l % 2], xT[(l + 1) % 2]
            # -------- stage A
            with tc.tile_pool(name=f"sA{l}", bufs=3) as ap_, \
                 tc.tile_pool(name=f"sAps{l}", bufs=2, space="PSUM") as aps:
                for ch in range(nchunk):
                    t = int(chunk_types[ch])
                    pj = aps.tile([P, 3 * D], dt.float32, tag="pj", name="pj")
                    nc.tensor.matmul(pj[:], xin[:, ch * P:(ch + 1) * P], pW[l, t][:],
                                     start=True, stop=True)
                    nc.vector.tensor_tensor(pj[:], pj[:], pB[l, t][:], op=ALU.add)
                    kvrow = ap_.tile([P, 128], dt.bfloat16, tag="kvrow", name="kvrow")
                    nc.vector.tensor_copy(kvrow[:, 0:64], pj[:, 0:64])
                    nc.vector.tensor_copy(kvrow[:, 64:128], pj[:, 128:192])
                    qrow = ap_.tile([P, 128], dt.bfloat16, tag="qrow", name="qrow")
                    nc.vector.tensor_copy(qrow[:, 0:64], pj[:, 64:128])
                    nqt = ap_.tile([P, D], dt.float32, tag="nqt", name="nqt")
                    nc.vector.tensor_tensor(nqt[:], pj[:, 64:128], afc1[l][:], op=ALU.mult)
                    nqr = ap_.tile([P, H], dt.float32, tag="nqr", name="nqr")
                    nc.vector.tensor_reduce(nqr[:], nqt[:].rearrange("p (h f) -> p h f", h=H),
                                            axis=mybir.AxisListType.X, op=ALU.add)
                    nc.vector.tensor_copy(qrow[:, 64:68], nqr[:])
                    nc.vector.memset(qrow[:, 68:128], 0.0)
                    nc.sync.dma_start(kv_shard[:, ch * 128:(ch + 1) * 128], kvrow[:])
                    nc.sync.dma_start(q_loc[ch * P:(ch + 1) * P, :], qrow[:])
            if DO_CC:
                nc.gpsimd.collective_compute(
                    "AllGather", mybir.AluOpType.bypass,
                    replica_groups=[list(range(NC_CORES))],
                    ins=[kv_shard[:]], outs=[kv_full[:]])
            else:
                nc.sync.dma_start(kv_full[0], kv_shard[:])
            for c in range(NC_CORES):
                nc.sync.dma_start(
                    kvtab[:, c * nchunk * 128:(c + 1) * nchunk * 128], kv_full[c])
            for pp_ in range(2):
                for par in range(2):
                    nc.vector.memset(acc[pp_, par][:], 0.0)

            # -------- edge pass
            if not DO_EDGE:
                pass
            else:
             with tc.tile_pool(name=f"eg{l}", bufs=3) as egp, \
                 tc.tile_pool(name=f"es{l}", bufs=2) as esp, \
                 tc.tile_pool(name=f"eps{l}", bufs=QBATCH, space="PSUM") as eps, \
                 tc.tile_pool(name=f"hps{l}", bufs=2, space="PSUM") as hps, \
                 tc.tile_pool(name=f"hbl{l}", bufs=3) as hbp:
                for r in range(R):
                    nsub_r = sum(int(pp["nsub"][r, s]) for s in range(1, SMAX + 1))
                    if nsub_r == 0:
                        continue
                    s_list = []
                    for s in range(1, SMAX + 1):
                        s_list += [s] * int(pp["nsub"][r, s])
                    sb0 = blk_start[r]
                    for g0 in range(0, nsub_r, GCALL_SUB):
                        gn = min(GCALL_SUB, nsub_r - g0)
                        hblk = hbp.tile([P, GCALL_SUB, D], dt.float32, tag="hblk", name="hblk")
                        gsl = slice((sb0 + g0) * 8, (sb0 + g0 + gn) * 8)
                        kvsl = egp.tile([P, 1, GCALL_SUB * P], dt.bfloat16, tag="kvsl", name="kvsl")
                        qsl = egp.tile([P, GCALL_SUB, 128], dt.bfloat16, tag="qsl", name="qsl")
                        if DO_GATHER:
                            nc.gpsimd.dma_gather(
                                out_ap=kvsl[:, :, 0:gn * P], in_ap=kvtab[:],
                                idxs_ap=kvix[:, gsl], num_idxs=gn * P,
                                num_idxs_reg=gn * P, elem_size=128, transpose=True,
                                sbuf_tokens_per_rank=P,
                                sbuf_free_dim_per_rank=256)
                            nc.gpsimd.dma_gather(
                                out_ap=qsl[:, 0:gn, :], in_ap=q_loc[:],
                                idxs_ap=qix[:, gsl], num_idxs=gn * P,
                                num_idxs_reg=gn * P, elem_size=128)
                        else:
                            nc.vector.memset(kvsl[:], 0.01)
                            nc.vector.memset(qsl[:], 0.01)
                        for q0 in range(0, gn, QBATCH):
                            qn = min(QBATCH, gn - q0)
                            sb_ao = esp.tile([P, QBATCH, D], dt.float32, tag="sb_ao", name="sb_ao")
                            sb_na = esp.tile([P, QBATCH, H], dt.float32, tag="sb_na", name="sb_na")
                            sb_ve = esp.tile([P, QBATCH, 68], dt.bfloat16, tag="sb_ve", name="sb_ve")
                            ps_list = []
                            for j in range(qn):
                                su = q0 + j
                                ps_tr = eps.tile([P, 132], dt.float32, tag="ps_tr", name="ps_tr")
                                ps_list.append(ps_tr)
                                nc.tensor.matmul(
                                    ps_tr[:], kvsl[:, 0, su * P:(su + 1) * P],
                                    TRt[l, r][:], start=True, stop=True)
                                nc.vector.tensor_tensor(
                                    sb_ao[:, j, :], ps_tr[:, 0:64],
                                    qsl[:, su, 0:64], op=ALU.mult)
                                # na' = wl*(a_dst*nq[dst] + afc2-dot); wl/a_dst
                                # prescaled into TR nk-block + a_dst_wl on host
                                nc.vector.scalar_tensor_tensor(
                                    sb_na[:, j, :], qsl[:, su, 64:68],
                                    float(scal["a_dst"][l][r]),
                                    ps_tr[:, 64:68], op0=ALU.mult, op1=ALU.add)
                            att = esp.tile([P, QBATCH, H], dt.float32, tag="att", name="att")
                            nc.vector.tensor_reduce(
                                att[:, 0:qn, :],
                                sb_ao[:, 0:qn, :].rearrange("p g (h f) -> p g h f", h=H),
                                axis=mybir.AxisListType.X, op=ALU.add)
                            tmp = esp.tile([P, QBATCH, H], dt.float32, tag="tmp", name="tmp")
                            nav = sb_na[:, 0:qn, :]
                            # wl*leaky(na) = max(0.01*na', na') since na' = wl*na
                            nc.vector.scalar_tensor_tensor(
                                tmp[:, 0:qn, :], nav, 0.01, nav,
                                op0=ALU.mult, op1=ALU.max)
                            nc.vector.tensor_tensor(tmp[:, 0:qn, :], tmp[:, 0:qn, :],
                                                    att[:, 0:qn, :], op=ALU.add)
                            nc.scalar.activation(sb_ve[:, 0:qn, 64:68], tmp[:, 0:qn, :],
                                                 AF.Exp, bias=c_zero[:, 0:1])
                            for j in range(qn):
                                su = q0 + j
                                s = s_list[g0 + su]
                                ps_tr = ps_list[j]
                                nc.vector.tensor_tensor(
                                    sb_ve[:, j, 0:64].rearrange("p (h f) -> p h f", h=H),
                                    ps_tr[:, 68:132].rearrange("p (h f) -> p h f", h=H),
                                    bch(sb_ve[:, j, 64:68]), op=ALU.mult)
                                ps_hte = hps.tile([P, 68], dt.float32, tag="ps_hte", name="ps_hte")
                                nc.tensor.matmul(ps_hte[:], Pst[s][:], sb_ve[:, j, :],
                                                 start=True, stop=True)
                                rc = esp.tile([P, H], dt.float32, tag="rc", name="rc")
                                nc.vector.reciprocal(rc[:], ps_hte[:, 64:68])
                                nc.vector.tensor_tensor(
                                    hblk[:, su, :].rearrange("p (h f) -> p h f", h=H),
                                    ps_hte[:, 0:64].rearrange("p (h f) -> p h f", h=H),
                                    bch(rc[:]), op=ALU.mult)
                        if DO_SCATTER:
                            nc.gpsimd.dma_scatter_add(
                                out_ap=acc[r % 2, 0][:],
                                out_ap_other=acc[r % 2, 1][:],
                                parity_reg=0,
                                sbuf_tokens_per_rank=P,
                                in_ap=hblk[:, 0:gn, :],
                                idxs_ap=scix[:, (sb0 + g0) * 8:(sb0 + g0 + gn) * 8],
                                num_idxs=gn * P, num_idxs_reg=gn * P, elem_size=D)

            # -------- stage D
            with tc.tile_pool(name=f"sD{l}", bufs=2) as dp, \
                 tc.tile_pool(name=f"sDps{l}", bufs=1, space="PSUM") as dps:
                for ch in range(nchunk):
                    t = int(chunk_types[ch])
                    par, m = ch % 2, ch // 2
                    ta = dp.tile([P, D], dt.float32, tag="ta", name="ta")
                    nc.vector.tensor_tensor(ta[:], acc[0, par][:, m * D:(m + 1) * D],
                                            acc[1, par][:, m * D:(m + 1) * D], op=ALU.add)
                    nc.vector.tensor_scalar(ta[:], ta[:], npi[:, ch:ch + 1], None,
                                            op0=ALU.mult)
                    tT_ps = dps.tile([D, P], dt.float32, tag="tT", name="tT")
                    nc.tensor.transpose(tT_ps[:], ta[:], ident[:])
                    tT = dp.tile([D, P], dt.float32, tag="tTsb", name="tTsb")
                    nc.vector.tensor_copy(tT[:], tT_ps[:])
                    tr = dps.tile([D, P], dt.float32, tag="tr", name="tr")
                    nc.tensor.matmul(tr[:], WaT[l, t][:], tT[:], start=True, stop=True)
                    out = dp.tile([D, P], dt.float32, tag="out", name="out")
                    a = float(scal["alpha"][l][t])
                    nc.vector.tensor_tensor(out[:], tr[:],
                                            baC[l, t][:].to_broadcast([D, P]), op=ALU.add)
                    nc.vector.tensor_scalar(out[:], out[:], a, None, op0=ALU.mult)
                    sk = dp.tile([D, P], dt.float32, tag="sk", name="sk")
                    nc.vector.tensor_scalar(sk[:], xin[:, ch * P:(ch + 1) * P],
                                            1.0 - a, None, op0=ALU.mult)
                    nc.vector.tensor_tensor(out[:], out[:], sk[:], op=ALU.add)
                    mu = dps.tile([1, P], dt.float32, tag="mu", name="mu")
                    nc.tensor.matmul(mu[:], onesMu[:], out[:], start=True, stop=True)
                    mu_sb = dp.tile([1, P], dt.float32, tag="mu_sb", name="mu_sb")
                    nc.vector.tensor_copy(mu_sb[:], mu[:])
                    muB = dps.tile([D, P], dt.float32, tag="muB", name="muB")
                    nc.tensor.matmul(muB[:], onesB[:], mu_sb[:], start=True, stop=True)
                    nc.vector.tensor_tensor(out[:], out[:], muB[:], op=ALU.subtract)
                    sq = dp.tile([D, P], dt.float32, tag="sq", name="sq")
                    nc.vector.tensor_tensor(sq[:], out[:], out[:], op=ALU.mult)
                    var = dps.tile([1, P], dt.float32, tag="var", name="var")
                    nc.tensor.matmul(var[:], onesMu[:], sq[:], start=True, stop=True)
                    sd = dp.tile([1, P], dt.float32, tag="sd", name="sd")
                    nc.scalar.activation(sd[:], var[:], AF.Sqrt, bias=c_eps[0:1, 0:1], scale=1.0)
                    rsd = dp.tile([1, P], dt.float32, tag="rsd", name="rsd")
                    nc.vector.reciprocal(rsd[:], sd[:])
                    rsdB = dps.tile([D, P], dt.float32, tag="rsdB", name="rsdB")
                    nc.tensor.matmul(rsdB[:], onesB[:], rsd[:], start=True, stop=True)
                    nc.vector.tensor_tensor(out[:], out[:], rsdB[:], op=ALU.mult)
                    nc.vector.tensor_scalar(out[:], out[:], lnG[l, t][:, 0:1], None,
                                            op0=ALU.mult)
                    nc.vector.tensor_tensor(xout[:, ch * P:(ch + 1) * P], out[:],
                                            lnB[l, t][:].to_broadcast([D, P]), op=ALU.add)

        # -------- output
        with tc.tile_pool(name="fin", bufs=3) as fp, \
             tc.tile_pool(name="finps", bufs=2, space="PSUM") as fps:
            xf = xT[L % 2]
            for ch in range(nchunk):
                xo_ps = fps.tile([P, D], dt.float32, tag="xo", name="xo")
                nc.tensor.transpose(xo_ps[:], xf[:, ch * P:(ch + 1) * P], ident[0:D, 0:D])
                xo = fp.tile([P, D], dt.float32, tag="xosb", name="xosb")
                nc.vector.tensor_copy(xo[:], xo_ps[:])
                nc.sync.dma_start(o_x[ch * P:(ch + 1) * P, :], xo[:])

    nc.finalize()
    return nc


# ---------------------------------------------------------------- packing
def _pack_inputs(pp, inputs, L):
    dn = pp["dn"]
    nchunk = dn // P
    f32 = np.float32
    h = np.asarray(inputs["h"], f32)
    new2old = pp["new2old"]

    afc = np.asarray(inputs["afc_w"], f32)
    rel_att = np.asarray(inputs["rel_att"], f32)
    rel_msg = np.asarray(inputs["rel_msg"], f32)
    rel_pri = np.asarray(inputs["rel_pri"], f32)
    nta = np.asarray(inputs["nta"], f32); nta1 = np.asarray(inputs["nta1"], f32)

    wl_all = [float(1.0 / (1.0 + np.exp(-np.asarray(inputs["wgt"], f32)[l])))
              for l in range(L)]
    TR = np.zeros((L, R, P, 132), f32)
    for l in range(L):
        for r in range(R):
            a_src = nta[l, SRC_NT[r]] * wl_all[l]
            for hh in range(H):
                pri = rel_pri[l, r, hh] / SQRT_DK
                TR[l, r, hh * DK:(hh + 1) * DK, hh * DK:(hh + 1) * DK] = rel_att[l, r, hh] * pri
                TR[l, r, hh * DK:(hh + 1) * DK, 64 + hh] = afc[l, DK:] * a_src
                TR[l, r, 64 + hh * DK:64 + (hh + 1) * DK, 68 + hh * DK:68 + (hh + 1) * DK] = rel_msg[l, r, hh]

    Ps = np.zeros((len(pp["classes"]), P, P), f32)
    for ci, s in enumerate(pp["classes"]):
        cap = P // s
        Ps[ci, :, cap:] = 1e-30  # pad seg columns: tiny-but-finite denominators
        for j in range(cap):
            Ps[ci, j * s:(j + 1) * s, j] = 1.0

    afc1rep = np.zeros((L, P, D), f32)
    for l in range(L):
        afc1rep[l, :, :] = np.tile(afc[l, :DK], H)[None, :]

    scal = {
        "a_dst": [[float(nta1[l, DST_NT[r]]) * wl_all[l] for r in range(R)] for l in range(L)],
        "wl": wl_all,
        "alpha": [[float(1.0 / (1.0 + np.exp(-np.asarray(inputs["skipp"], f32)[l, t])))
                   for t in range(T)] for l in range(L)],
    }

    Wk = np.asarray(inputs["Wk"], f32); Wq = np.asarray(inputs["Wq"], f32)
    Wv = np.asarray(inputs["Wv"], f32)
    bk = np.asarray(inputs["bk"], f32); bq = np.asarray(inputs["bq"], f32)
    bv = np.asarray(inputs["bv"], f32)
    pWa = np.concatenate([Wk, Wq, Wv], axis=-1)[:L]
    pBa = np.concatenate([bk, bq, bv], axis=-1)[:L]
    pB_rep = np.broadcast_to(pBa[:, :, None, :], (L, T, P, 3 * D)).copy()

    shared = {
        "adW": np.asarray(inputs["adapt_W"], f32),
        "adB": np.asarray(inputs["adapt_b"], f32)[:, :, None],
        "pW": np.ascontiguousarray(pWa), "pB": pB_rep,
        "WaT": np.ascontiguousarray(np.asarray(inputs["Wa"], f32)[:L]),
        "baC": np.ascontiguousarray(np.asarray(inputs["ba"], f32)[:L][..., None]),
        "lnG": np.ascontiguousarray(np.asarray(inputs["ln_g"], f32)[:L][..., None]),
        "lnB": np.ascontiguousarray(np.asarray(inputs["ln_b"], f32)[:L][..., None]),
        "TRr": _bf16(TR), "Ps": _bf16(Ps),
        "afc1rep": afc1rep,
        "ident": np.eye(P, dtype=f32),
    }

    in_maps = []
    for c in range(NC_CORES):
        co = pp["cores"][c]
        hs = np.zeros((dn, D_IN), f32)
        ids = new2old[c * dn:(c + 1) * dn]
        rl = ids >= 0
        hs[rl] = h[ids[rl]]
        npi = (1.0 / pp["npres"][c]).reshape(nchunk, P).T
        m = dict(shared)
        m["h_shard"] = hs
        m["npres_inv"] = np.ascontiguousarray(npi)
        m["kv_idx"] = _wrap16(co["e_src"])
        m["q_idx"] = _wrap16(co["e_dstl"])
        m["sc_idx"] = _wrap16(co["seg_dstl"])
        in_maps.append(m)
    return in_maps, scal


# ---------------------------------------------------------------- entry
def kernel(**inputs):
    from concourse.bass_utils import run_bass_kernel_spmd

    L = int(os.environ.get("BRING_L", L_FULL))
    src = np.asarray(inputs["src"]); dst = np.asarray(inputs["dst"])
    etype = np.asarray(inputs["etype"]); ntype = np.asarray(inputs["ntype"])
    pp = _prep(src, dst, etype, ntype)
    in_maps, scal = _pack_inputs(pp, inputs, L)

    key = (L, pp["dn"], tuple(pp["nsub"].ravel().tolist()),
           tuple(pp["chunk_types"].tolist()),
           tuple(map(tuple, scal["a_dst"])), tuple(scal["wl"]),
           tuple(map(tuple, scal["alpha"])))
    if key not in _PROG_CACHE:
        _PROG_CACHE[key] = _build(pp, L, scal)
    nc = _PROG_CACHE[key]

    res = run_bass_kernel_spmd(
        nc, in_maps, list(range(NC_CORES)),
        trace=bool(int(os.environ.get("BASS_KERNEL_TRACE", "0"))))
    kernel.last_results = res
    full = np.concatenate([res.results[c]["x_out"] for c in range(NC_CORES)], axis=0)
    return full[pp["old2new"]].astype(np.float32)



# revision 23
# speedup vs baseline: 1.4190x; 1.4190x over previous
"""Trainium2 Bass kernel for nn_DevignModel (heterogeneous GNN message passing).

Self-contained: host preprocessing (node relabel by type, 8-way dst sharding,
(etype, dst)-sorted edges packed into segment-size classes) + Bass/Tile SPMD
program for 8 NeuronCores + output assembly.

Per layer on device:
  stage A: per 128-node chunk: K/Q/V projections (PE), nq head-dot, bf16 node
           tables -> DRAM; AllGather of the [k|v] table across the 8 cores.
  edge pass: dma_gather(transpose) of [k|v] rows -> feature-major stationary
           operand; one matmul vs per-relation rhs [A*pri | afc2*a_src | M]
           -> edge-major K~/nk~/V~; attention dot + leaky-relu gate; exp;
           segment softmax + message aggregation via constant P_s matmuls;
           dma_scatter_add of per-segment messages into per-node accumulators
           (parity ping-pong tables to avoid RMW races).
  stage D: cross-etype mean, Wa skip-mix, LayerNorm (feature-major).
"""
import os
import numpy as np
import ml_dtypes

D_IN, D, H, DK, L_FULL, T, R = 128, 64, 4, 16, 4, 3, 32
SQRT_DK = 4.0
NC_CORES = 8
P = 128
SMAX = 16
GCALL_SUB = 7   # subtiles per gather call (896 idxs; SWDGE ring limit — 1792 crashes)
SCALL_SUB = 4    # subtiles per scatter call (512 idxs)
QBATCH = 4       # subtiles batched per PSUM group

SRC_NT = np.array([0 if e <= 9 else (1 if e <= 21 else 2) for e in range(R)], dtype=np.int64)
def _dst_nt(e):
    if e <= 2 or 10 <= e <= 13 or 22 <= e <= 24: return 0
    if 3 <= e <= 6 or 14 <= e <= 17 or 25 <= e <= 28: return 1
    return 2
DST_NT = np.array([_dst_nt(e) for e in range(R)], dtype=np.int64)


def _wrap16(ix):
    """dma_gather/scatter idx layout: element i at [i%16, i//16], replicated
    across the 8 q7 cores (128 partitions)."""
    ix = np.asarray(ix, np.int16)
    out = ix.reshape(len(ix) // 16, 16).T.copy()
    return np.tile(out, (8, 1))


def _bf16(x):
    return np.asarray(x, np.float32).astype(ml_dtypes.bfloat16)


# ----------------------------------------------------------------- host prep
def _prep(src, dst, etype, ntype):
    N = len(ntype)
    order = np.argsort(ntype, kind="stable")
    # deal each type's nodes round-robin across shards -> balanced type mix
    raw_shards = [[] for _ in range(NC_CORES)]
    for t in range(T):
        ids_t = order[np.asarray(ntype)[order] == t]
        for c in range(NC_CORES):
            raw_shards[c].append(ids_t[c::NC_CORES])
    raw_shards = [np.concatenate(s) for s in raw_shards]
    nch = np.zeros(T, np.int64)
    for c in range(NC_CORES):
        tys = ntype[raw_shards[c]]
        for t in range(T):
            nch[t] = max(nch[t], -(-int((tys == t).sum()) // P))
    nch[T - 1] += 1  # guaranteed dummy chunk per shard
    chunk_types = np.concatenate([np.full(nch[t], t, np.int64) for t in range(T)])
    dn = int(nch.sum()) * P
    N_pad = dn * NC_CORES

    new2old = np.full(N_pad, -1, np.int64)
    for c in range(NC_CORES):
        ids = raw_shards[c]; tys = ntype[ids]
        off = c * dn
        for t in range(T):
            sel = ids[tys == t]
            new2old[off:off + len(sel)] = sel
            off += int(nch[t]) * P
    old2new = np.full(N, -1, np.int64)
    real = new2old >= 0
    old2new[new2old[real]] = np.nonzero(real)[0]
    assert (old2new >= 0).all()

    src_n = old2new[np.asarray(src)]
    dst_n = old2new[np.asarray(dst)]
    et = np.asarray(etype, np.int64)
    core_of = dst_n // dn

    per_core = []
    for c in range(NC_CORES):
        em = np.nonzero(core_of == c)[0]
        dstl = dst_n[em] - c * dn
        bet = et[em]
        o = np.lexsort((dstl, bet))
        em, dstl, bet = em[o], dstl[o], bet[o]
        blocks = []
        for r in range(R):
            sel = np.nonzero(bet == r)[0]
            bd, be = dstl[sel], em[sel]
            if len(sel):
                cut = np.nonzero(np.diff(bd))[0] + 1
                starts = np.concatenate([[0], cut]); ends = np.concatenate([cut, [len(bd)]])
            else:
                starts = ends = np.array([], np.int64)
            assert (ends - starts).max(initial=0) <= SMAX
            cls = {}
            for s0, e0 in zip(starts, ends):
                cls.setdefault(int(e0 - s0), []).append((int(s0), int(e0)))
            blocks.append({"cls": cls, "dstl": bd})
        per_core.append(blocks)

    nsub = np.zeros((R, SMAX + 1), np.int64)
    for c in range(NC_CORES):
        for r in range(R):
            for s, lst in per_core[c][r]["cls"].items():
                nsub[r, s] = max(nsub[r, s], -(-len(lst) // (P // s)))

    cores = []
    for c in range(NC_CORES):
        dummy = dn - 1
        e_src, e_dstl, seg_dstl = [], [], []
        for r in range(R):
            b = per_core[c][r]
            for s in range(1, SMAX + 1):
                if nsub[r, s] == 0:
                    continue
                cap = P // s
                segs = b["cls"].get(s, [])
                for ti in range(int(nsub[r, s])):
                    for k in range(cap):
                        gi = ti * cap + k
                        if gi < len(segs):
                            s0, e0 = segs[gi]
                            dl = int(b["dstl"][s0])
                            e_dstl.extend([dl] * s)
                            seg_dstl.append(dl)
                        else:
                            e_dstl.extend([dummy] * s)
                            seg_dstl.append(dummy)
                    tail = P - cap * s
                    e_dstl.extend([dummy] * tail)
                    seg_dstl.extend([dummy] * (P - cap))
        cores.append({"e_dstl": np.array(e_dstl, np.int64),
                      "seg_dstl": np.array(seg_dstl, np.int64)})

    # second pass: e_src needs the per-seg edge id ranges (kept separately to
    # avoid storing eids in blocks twice)
    for c in range(NC_CORES):
        em = np.nonzero(core_of == c)[0]
        dstl = dst_n[em] - c * dn
        bet = et[em]
        o = np.lexsort((dstl, bet))
        em, dstl, bet = em[o], dstl[o], bet[o]
        dummy_g = c * dn + (dn - 1)
        e_src = []
        for r in range(R):
            sel = np.nonzero(bet == r)[0]
            bd, be = dstl[sel], em[sel]
            if len(sel):
                cut = np.nonzero(np.diff(bd))[0] + 1
                starts = np.concatenate([[0], cut]); ends = np.concatenate([cut, [len(bd)]])
            else:
                starts = ends = np.array([], np.int64)
            cls = {}
            for s0, e0 in zip(starts, ends):
                cls.setdefault(int(e0 - s0), []).append((int(s0), int(e0)))
            for s in range(1, SMAX + 1):
                if nsub[r, s] == 0:
                    continue
                cap = P // s
                segs = cls.get(s, [])
                for ti in range(int(nsub[r, s])):
                    for k in range(cap):
                        gi = ti * cap + k
                        if gi < len(segs):
                            s0, e0 = segs[gi]
                            e_src.extend(src_n[be[s0:e0]].tolist())
                        else:
                            e_src.extend([dummy_g] * s)
                    e_src.extend([dummy_g] * (P - cap * s))
        cores[c]["e_src"] = np.array(e_src, np.int64)
        assert len(cores[c]["e_src"]) == len(cores[c]["e_dstl"])

    sub_meta = []
    for r in range(R):
        for s in range(1, SMAX + 1):
            sub_meta += [(r, s)] * int(nsub[r, s])
    classes = sorted({s for (_, s) in sub_meta})

    npres = np.ones((NC_CORES, dn), np.float32)
    seg_all = np.unique(dst_n * R + et)
    cnt = np.zeros(N_pad, np.float32)
    np.add.at(cnt, seg_all // R, 1.0)
    for c in range(NC_CORES):
        npres[c] = np.maximum(cnt[c * dn:(c + 1) * dn], 1.0)

    return {"dn": dn, "N_pad": N_pad, "new2old": new2old, "old2new": old2new,
            "chunk_types": chunk_types, "cores": cores, "npres": npres,
            "nsub": nsub, "sub_meta": sub_meta, "classes": classes}


# ------------------------------------------------------------- program build
_PROG_CACHE = {}


def _build(pp, L, scal):
    DO_EDGE = os.environ.get("K_EDGE", "1") != "0"
    DO_GATHER = os.environ.get("K_GATHER", "1") != "0"
    DO_SCATTER = os.environ.get("K_SCATTER", "1") != "0"
    DO_CC = os.environ.get("K_CC", "1") != "0"

    import concourse.bass as bass
    import concourse.bacc as bacc
    import concourse.mybir as mybir
    import concourse.tile as tile
    from contextlib import ExitStack

    dt = mybir.dt
    AF = mybir.ActivationFunctionType
    ALU = mybir.AluOpType

    dn, N_pad = pp["dn"], pp["N_pad"]
    nchunk = dn // P
    chunk_types = pp["chunk_types"]
    sub_meta = pp["sub_meta"]
    classes = pp["classes"]
    nsubtot = len(sub_meta)
    nslots = nsubtot * P

    nc = bacc.Bacc("TRN2", target_bir_lowering=False, debug=False,
                   num_devices=NC_CORES, dynamic_dma_scratch_size=16384)

    p_h = nc.dram_tensor("h_shard", [dn, D_IN], dt.float32, kind="ExternalInput")
    p_adW = nc.dram_tensor("adW", [T, D_IN, D], dt.float32, kind="ExternalInput")
    p_adB = nc.dram_tensor("adB", [T, D, 1], dt.float32, kind="ExternalInput")
    p_pW = nc.dram_tensor("pW", [L, T, D, 3 * D], dt.float32, kind="ExternalInput")
    p_pB = nc.dram_tensor("pB", [L, T, P, 3 * D], dt.float32, kind="ExternalInput")
    p_WaT = nc.dram_tensor("WaT", [L, T, D, D], dt.float32, kind="ExternalInput")
    p_baC = nc.dram_tensor("baC", [L, T, D, 1], dt.float32, kind="ExternalInput")
    p_lnG = nc.dram_tensor("lnG", [L, T, D, 1], dt.float32, kind="ExternalInput")
    p_lnB = nc.dram_tensor("lnB", [L, T, D, 1], dt.float32, kind="ExternalInput")
    p_TR = nc.dram_tensor("TRr", [L, R, P, 132], dt.bfloat16, kind="ExternalInput")
    p_Ps = nc.dram_tensor("Ps", [len(classes), P, P], dt.bfloat16, kind="ExternalInput")
    p_afc1 = nc.dram_tensor("afc1rep", [L, P, D], dt.float32, kind="ExternalInput")
    p_ident = nc.dram_tensor("ident", [P, P], dt.float32, kind="ExternalInput")
    p_npi = nc.dram_tensor("npres_inv", [P, nchunk], dt.float32, kind="ExternalInput")
    p_kvix = nc.dram_tensor("kv_idx", [P, nslots // 16], dt.int16, kind="ExternalInput")
    p_qix = nc.dram_tensor("q_idx", [P, nslots // 16], dt.int16, kind="ExternalInput")
    p_scix = nc.dram_tensor("sc_idx", [P, nslots // 16], dt.int16, kind="ExternalInput")

    o_x = nc.dram_tensor("x_out", [dn, D], dt.float32, kind="ExternalOutput")

    # kv stripes layout: [128 parts, nchunk, 128 feats]; node ch*128+p at [p, ch, :]
    kv_shard = nc.dram_tensor("kv_shard", [P, nchunk * 128], dt.bfloat16)
    kv_full = nc.dram_tensor("kv_full", [NC_CORES, P, nchunk * 128], dt.bfloat16,
                             addr_space="Shared")
    q_loc = nc.dram_tensor("q_loc", [dn, 128], dt.bfloat16)
    nacc = -(-nchunk // 2)  # accumulator free-dim blocks per parity

    def bch(apx, n=DK):
        """[p, H] -> [p, H, n] broadcast (step-0 inner dim)."""
        return apx.to_broadcast([apx.shape[0], H, n])

    with tile.TileContext(nc) as tc, ExitStack() as ST:
        cpool = ST.enter_context(tc.tile_pool(name="consts", bufs=1))
        ident = cpool.tile([P, P], dt.float32)
        nc.sync.dma_start(ident[:], p_ident[:])
        adW, adB = {}, {}
        for t in range(T):
            adW[t] = cpool.tile([D_IN, D], dt.float32, tag=f"adW{t}", name=f"adW{t}")
            nc.sync.dma_start(adW[t][:], p_adW[t])
            adB[t] = cpool.tile([D, 1], dt.float32, tag=f"adB{t}", name=f"adB{t}")
            nc.sync.dma_start(adB[t][:], p_adB[t])
        pW, pB, WaT, baC, lnG, lnB = {}, {}, {}, {}, {}, {}
        for l in range(L):
            for t in range(T):
                for nm, store, par, shp in (
                        ("pW", pW, p_pW, [D, 3 * D]), ("pB", pB, p_pB, [P, 3 * D]),
                        ("Wa", WaT, p_WaT, [D, D]), ("ba", baC, p_baC, [D, 1]),
                        ("lg", lnG, p_lnG, [D, 1]), ("lb", lnB, p_lnB, [D, 1])):
                    store[l, t] = cpool.tile(shp, dt.float32, tag=f"{nm}{l}_{t}", name=f"{nm}{l}_{t}")
                    nc.sync.dma_start(store[l, t][:], par[l, t])
        TRt = {}
        for l in range(L):
            for r in range(R):
                TRt[l, r] = cpool.tile([P, 132], dt.bfloat16, tag=f"TR{l}_{r}", name=f"TR{l}_{r}")
                nc.sync.dma_start(TRt[l, r][:], p_TR[l, r])
        Pst = {}
        for ci, s in enumerate(classes):
            Pst[s] = cpool.tile([P, P], dt.bfloat16, tag=f"Ps{s}", name=f"Ps{s}")
            nc.sync.dma_start(Pst[s][:], p_Ps[ci])
        afc1 = {}
        for l in range(L):
            afc1[l] = cpool.tile([P, D], dt.float32, tag=f"afc1_{l}", name=f"afc1_{l}")
            nc.sync.dma_start(afc1[l][:], p_afc1[l])
        npi = cpool.tile([P, nchunk], dt.float32)
        nc.sync.dma_start(npi[:], p_npi[:])
        kvix = cpool.tile([P, nslots // 16], dt.int16, tag="kvix", name="kvix")
        nc.sync.dma_start(kvix[:], p_kvix[:])
        qix = cpool.tile([P, nslots // 16], dt.int16, tag="qix", name="qix")
        nc.sync.dma_start(qix[:], p_qix[:])
        scix = cpool.tile([P, nslots // 16], dt.int16, tag="scix", name="scix")
        nc.sync.dma_start(scix[:], p_scix[:])
        onesMu = cpool.tile([D, 1], dt.float32, tag="onesMu", name="onesMu")
        nc.vector.memset(onesMu[:], 1.0 / D)
        onesB = cpool.tile([1, D], dt.float32, tag="onesB", name="onesB")
        nc.vector.memset(onesB[:], 1.0)
        zero64 = cpool.tile([P, D], dt.float32, tag="zero64", name="zero64")
        nc.vector.memset(zero64[:], 0.0)
        c_zero = cpool.tile([P, 1], dt.float32, tag="c_zero", name="c_zero")
        nc.vector.memset(c_zero[:], 0.0)
        c_eps = cpool.tile([1, 1], dt.float32, tag="c_eps", name="c_eps")
        nc.vector.memset(c_eps[:], 1e-5)
        xT = [cpool.tile([D, dn], dt.float32, tag=f"xT{i}", name=f"xT{i}") for i in range(2)]
        # SBUF kv table (gather source) + SBUF scatter accumulators
        kvtab = cpool.tile([P, NC_CORES * nchunk * 128], dt.bfloat16, tag="kvtab", name="kvtab")
        acc = {}
        for pp_ in range(2):          # relation ping-pong
            for par in range(2):      # stripe parity (even/odd chunk)
                acc[pp_, par] = cpool.tile([P, nacc * D], dt.float32,
                                           tag=f"acc{pp_}_{par}", name=f"acc{pp_}_{par}")

        # -------- stage 0: adapt
        with tc.tile_pool(name="s0", bufs=3) as s0p, \
             tc.tile_pool(name="s0ps", bufs=2, space="PSUM") as s0ps:
            for ch in range(nchunk):
                t = int(chunk_types[ch])
                hch = s0p.tile([P, D_IN], dt.float32, tag="hch", name="hch")
                nc.sync.dma_start(hch[:], p_h[ch * P:(ch + 1) * P, :])
                hT_ps = s0ps.tile([P, P], dt.float32, tag="hT", name="hT")
                nc.tensor.transpose(hT_ps[:], hch[:], ident[:])
                hT = s0p.tile([D_IN, P], dt.float32, tag="hTsb", name="hTsb")
                nc.vector.tensor_copy(hT[:], hT_ps[:])
                x0 = s0ps.tile([D, P], dt.float32, tag="x0", name="x0")
                nc.tensor.matmul(x0[:], adW[t][:], hT[:], start=True, stop=True)
                nc.scalar.activation(xT[0][:, ch * P:(ch + 1) * P], x0[:],
                                     AF.Tanh, bias=adB[t][:, 0:1], scale=1.0)

        blk_start = {}
        pos = 0
        for r in range(R):
            blk_start[r] = pos
            pos += sum(int(pp["nsub"][r, s]) for s in range(1, SMAX + 1))

        for l in range(L):
            xin, xout = xT[l % 2], xT[(l + 1) % 2]
            # -------- stage A
            with tc.tile_pool(name=f"sA{l}", bufs=3) as ap_, \
                 tc.tile_pool(name=f"sAps{l}", bufs=2, space="PSUM") as aps:
                for ch in range(nchunk):
                    t = int(chunk_types[ch])
                    pj = aps.tile([P, 3 * D], dt.float32, tag="pj", name="pj")
                    nc.tensor.matmul(pj[:], xin[:, ch * P:(ch + 1) * P], pW[l, t][:],
                                     start=True, stop=True)
                    nc.vector.tensor_tensor(pj[:], pj[:], pB[l, t][:], op=ALU.add)
                    kvrow = ap_.tile([P, 128], dt.bfloat16, tag="kvrow", name="kvrow")
                    nc.vector.tensor_copy(kvrow[:, 0:64], pj[:, 0:64])
                    nc.vector.tensor_copy(kvrow[:, 64:128], pj[:, 128:192])
                    qrow = ap_.tile([P, 128], dt.bfloat16, tag="qrow", name="qrow")
                    nc.vector.tensor_copy(qrow[:, 0:64], pj[:, 64:128])
                    nqt = ap_.tile([P, D], dt.float32, tag="nqt", name="nqt")
                    nc.vector.tensor_tensor(nqt[:], pj[:, 64:128], afc1[l][:], op=ALU.mult)
                    nqr = ap_.tile([P, H], dt.float32, tag="nqr", name="nqr")
                    nc.vector.tensor_reduce(nqr[:], nqt[:].rearrange("p (h f) -> p h f", h=H),
                                            axis=mybir.AxisListType.X, op=ALU.add)
                    nc.vector.tensor_copy(qrow[:, 64:68], nqr[:])
                    nc.vector.memset(qrow[:, 68:128], 0.0)
                    nc.sync.dma_start(kv_shard[:, ch * 128:(ch + 1) * 128], kvrow[:])
                    nc.sync.dma_start(q_loc[ch * P:(ch + 1) * P, :], qrow[:])
            if DO_CC:
                nc.gpsimd.collective_compute(
                    "AllGather", mybir.AluOpType.bypass,
                    replica_groups=[list(range(NC_CORES))],
                    ins=[kv_shard[:]], outs=[kv_full[:]])
            else:
                nc.sync.dma_start(kv_full[0], kv_shard[:])
            for c in range(NC_CORES):
                nc.sync.dma_start(
                    kvtab[:, c * nchunk * 128:(c + 1) * nchunk * 128], kv_full[c])
            for pp_ in range(2):
                for par in range(2):
                    nc.vector.memset(acc[pp_, par][:], 0.0)

            # -------- edge pass
            if not DO_EDGE:
                pass
            else:
             with tc.tile_pool(name=f"eg{l}", bufs=2) as egp, \
                 tc.tile_pool(name=f"es{l}", bufs=2) as esp, \
                 tc.tile_pool(name=f"eps{l}", bufs=QBATCH, space="PSUM") as eps, \
                 tc.tile_pool(name=f"hps{l}", bufs=2, space="PSUM") as hps, \
                 tc.tile_pool(name=f"hbl{l}", bufs=2) as hbp:
                for r in range(R):
                    nsub_r = sum(int(pp["nsub"][r, s]) for s in range(1, SMAX + 1))
                    if nsub_r == 0:
                        continue
                    s_list = []
                    for s in range(1, SMAX + 1):
                        s_list += [s] * int(pp["nsub"][r, s])
                    sb0 = blk_start[r]
                    hblk = hbp.tile([P, nsub_r, D], dt.float32, tag="hblk", name="hblk")
                    for g0 in range(0, nsub_r, GCALL_SUB):
                        gn = min(GCALL_SUB, nsub_r - g0)
                        gsl = slice((sb0 + g0) * 8, (sb0 + g0 + gn) * 8)
                        kvsl = egp.tile([P, 1, GCALL_SUB * P], dt.bfloat16, tag="kvsl", name="kvsl")
                        qsl = egp.tile([P, GCALL_SUB, 128], dt.bfloat16, tag="qsl", name="qsl")
                        if DO_GATHER:
                            nc.gpsimd.dma_gather(
                                out_ap=kvsl[:, :, 0:gn * P], in_ap=kvtab[:],
                                idxs_ap=kvix[:, gsl], num_idxs=gn * P,
                                num_idxs_reg=gn * P, elem_size=128, transpose=True,
                                sbuf_tokens_per_rank=P,
                                sbuf_free_dim_per_rank=256)
                            nc.gpsimd.dma_gather(
                                out_ap=qsl[:, 0:gn, :], in_ap=q_loc[:],
                                idxs_ap=qix[:, gsl], num_idxs=gn * P,
                                num_idxs_reg=gn * P, elem_size=128)
                        else:
                            nc.vector.memset(kvsl[:], 0.01)
                            nc.vector.memset(qsl[:], 0.01)
                        for q0 in range(0, gn, QBATCH):
                            qn = min(QBATCH, gn - q0)
                            sb_ao = esp.tile([P, QBATCH, D], dt.float32, tag="sb_ao", name="sb_ao")
                            sb_na = esp.tile([P, QBATCH, H], dt.float32, tag="sb_na", name="sb_na")
                            sb_ve = esp.tile([P, QBATCH, 68], dt.bfloat16, tag="sb_ve", name="sb_ve")
                            ps_list = []
                            for j in range(qn):
                                su = q0 + j
                                ps_tr = eps.tile([P, 132], dt.float32, tag="ps_tr", name="ps_tr")
                                ps_list.append(ps_tr)
                                nc.tensor.matmul(
                                    ps_tr[:], kvsl[:, 0, su * P:(su + 1) * P],
                                    TRt[l, r][:], start=True, stop=True)
                                nc.vector.tensor_tensor(
                                    sb_ao[:, j, :], ps_tr[:, 0:64],
                                    qsl[:, su, 0:64], op=ALU.mult)
                                # na' = wl*(a_dst*nq[dst] + afc2-dot); wl/a_dst
                                # prescaled into TR nk-block + a_dst_wl on host
                                nc.vector.scalar_tensor_tensor(
                                    sb_na[:, j, :], qsl[:, su, 64:68],
                                    float(scal["a_dst"][l][r]),
                                    ps_tr[:, 64:68], op0=ALU.mult, op1=ALU.add)
                            att = esp.tile([P, QBATCH, H], dt.float32, tag="att", name="att")
                            nc.vector.tensor_reduce(
                                att[:, 0:qn, :],
                                sb_ao[:, 0:qn, :].rearrange("p g (h f) -> p g h f", h=H),
                                axis=mybir.AxisListType.X, op=ALU.add)
                            tmp = esp.tile([P, QBATCH, H], dt.float32, tag="tmp", name="tmp")
                            nav = sb_na[:, 0:qn, :]
                            # wl*leaky(na) = max(0.01*na', na') since na' = wl*na
                            nc.vector.scalar_tensor_tensor(
                                tmp[:, 0:qn, :], nav, 0.01, nav,
                                op0=ALU.mult, op1=ALU.max)
                            nc.vector.tensor_tensor(tmp[:, 0:qn, :], tmp[:, 0:qn, :],
                                                    att[:, 0:qn, :], op=ALU.add)
                            nc.scalar.activation(sb_ve[:, 0:qn, 64:68], tmp[:, 0:qn, :],
                                                 AF.Exp, bias=c_zero[:, 0:1])
                            for j in range(qn):
                                su = q0 + j
                                s = s_list[g0 + su]
                                ps_tr = ps_list[j]
                                nc.vector.tensor_tensor(
                                    sb_ve[:, j, 0:64].rearrange("p (h f) -> p h f", h=H),
                                    ps_tr[:, 68:132].rearrange("p (h f) -> p h f", h=H),
                                    bch(sb_ve[:, j, 64:68]), op=ALU.mult)
                                ps_hte = hps.tile([P, 68], dt.float32, tag="ps_hte", name="ps_hte")
                                nc.tensor.matmul(ps_hte[:], Pst[s][:], sb_ve[:, j, :],
                                                 start=True, stop=True)
                                rc = esp.tile([P, H], dt.float32, tag="rc", name="rc")
                                nc.vector.reciprocal(rc[:], ps_hte[:, 64:68])
                                nc.vector.tensor_tensor(
                                    hblk[:, g0 + su, :].rearrange("p (h f) -> p h f", h=H),
                                    ps_hte[:, 0:64].rearrange("p (h f) -> p h f", h=H),
                                    bch(rc[:]), op=ALU.mult)
                    for g0 in (range(0, nsub_r, SCALL_SUB) if DO_SCATTER else []):
                        gn = min(SCALL_SUB, nsub_r - g0)
                        nc.gpsimd.dma_scatter_add(
                            out_ap=acc[r % 2, 0][:],
                            out_ap_other=acc[r % 2, 1][:],
                            parity_reg=0,
                            sbuf_tokens_per_rank=P,
                            in_ap=hblk[:, g0:g0 + gn, :],
                            idxs_ap=scix[:, (sb0 + g0) * 8:(sb0 + g0 + gn) * 8],
                            num_idxs=gn * P, num_idxs_reg=gn * P, elem_size=D)

            # -------- stage D
            with tc.tile_pool(name=f"sD{l}", bufs=2) as dp, \
                 tc.tile_pool(name=f"sDps{l}", bufs=1, space="PSUM") as dps:
                for ch in range(nchunk):
                    t = int(chunk_types[ch])
                    par, m = ch % 2, ch // 2
                    ta = dp.tile([P, D], dt.float32, tag="ta", name="ta")
                    nc.vector.tensor_tensor(ta[:], acc[0, par][:, m * D:(m + 1) * D],
                                            acc[1, par][:, m * D:(m + 1) * D], op=ALU.add)
                    nc.vector.tensor_scalar(ta[:], ta[:], npi[:, ch:ch + 1], None,
                                            op0=ALU.mult)
                    tT_ps = dps.tile([D, P], dt.float32, tag="tT", name="tT")
                    nc.tensor.transpose(tT_ps[:], ta[:], ident[:])
                    tT = dp.tile([D, P], dt.float32, tag="tTsb", name="tTsb")
                    nc.vector.tensor_copy(tT[:], tT_ps[:])
                    tr = dps.tile([D, P], dt.float32, tag="tr", name="tr")
                    nc.tensor.matmul(tr[:], WaT[l, t][:], tT[:], start=True, stop=True)
                    out = dp.tile([D, P], dt.float32, tag="out", name="out")
                    a = float(scal["alpha"][l][t])
                    nc.vector.tensor_tensor(out[:], tr[:],
                                            baC[l, t][:].to_broadcast([D, P]), op=ALU.add)
                    nc.vector.tensor_scalar(out[:], out[:], a, None, op0=ALU.mult)
                    sk = dp.tile([D, P], dt.float32, tag="sk", name="sk")
                    nc.vector.tensor_scalar(sk[:], xin[:, ch * P:(ch + 1) * P],
                                            1.0 - a, None, op0=ALU.mult)
                    nc.vector.tensor_tensor(out[:], out[:], sk[:], op=ALU.add)
                    mu = dps.tile([1, P], dt.float32, tag="mu", name="mu")
                    nc.tensor.matmul(mu[:], onesMu[:], out[:], start=True, stop=True)
                    mu_sb = dp.tile([1, P], dt.float32, tag="mu_sb", name="mu_sb")
                    nc.vector.tensor_copy(mu_sb[:], mu[:])
                    muB = dps.tile([D, P], dt.float32, tag="muB", name="muB")
                    nc.tensor.matmul(muB[:], onesB[:], mu_sb[:], start=True, stop=True)
                    nc.vector.tensor_tensor(out[:], out[:], muB[:], op=ALU.subtract)
                    sq = dp.tile([D, P], dt.float32, tag="sq", name="sq")
                    nc.vector.tensor_tensor(sq[:], out[:], out[:], op=ALU.mult)
                    var = dps.tile([1, P], dt.float32, tag="var", name="var")
                    nc.tensor.matmul(var[:], onesMu[:], sq[:], start=True, stop=True)
                    sd = dp.tile([1, P], dt.float32, tag="sd", name="sd")
                    nc.scalar.activation(sd[:], var[:], AF.Sqrt, bias=c_eps[0:1, 0:1], scale=1.0)
                    rsd = dp.tile([1, P], dt.float32, tag="rsd", name="rsd")
                    nc.vector.reciprocal(rsd[:], sd[:])
                    rsdB = dps.tile([D, P], dt.float32, tag="rsdB", name="rsdB")
                    nc.tensor.matmul(rsdB[:], onesB[:], rsd[:], start=True, stop=True)
                    nc.vector.tensor_tensor(out[:], out[:], rsdB[:], op=ALU.mult)
                    nc.vector.tensor_scalar(out[:], out[:], lnG[l, t][:, 0:1], None,
                                            op0=ALU.mult)
                    nc.vector.tensor_tensor(xout[:, ch * P:(ch + 1) * P], out[:],
                                            lnB[l, t][:].to_broadcast([D, P]), op=ALU.add)

        # -------- output
        with tc.tile_pool(name="fin", bufs=3) as fp, \
             tc.tile_pool(name="finps", bufs=2, space="PSUM") as fps:
            xf = xT[L % 2]
            for ch in range(nchunk):
                xo_ps = fps.tile([P, D], dt.float32, tag="xo", name="xo")
                nc.tensor.transpose(xo_ps[:], xf[:, ch * P:(ch + 1) * P], ident[0:D, 0:D])
                xo = fp.tile([P, D], dt.float32, tag="xosb", name="xosb")
                nc.vector.tensor_copy(xo[:], xo_ps[:])
                nc.sync.dma_start(o_x[ch * P:(ch + 1) * P, :], xo[:])

    nc.finalize()
    return nc


# ---------------------------------------------------------------- packing
def _pack_inputs(pp, inputs, L):
    dn = pp["dn"]
    nchunk = dn // P
    f32 = np.float32
    h = np.asarray(inputs["h"], f32)
    new2old = pp["new2old"]

    afc = np.asarray(inputs["afc_w"], f32)
    rel_att = np.asarray(inputs["rel_att"], f32)
    rel_msg = np.asarray(inputs["rel_msg"], f32)
    rel_pri = np.asarray(inputs["rel_pri"], f32)
    nta = np.asarray(inputs["nta"], f32); nta1 = np.asarray(inputs["nta1"], f32)

    wl_all = [float(1.0 / (1.0 + np.exp(-np.asarray(inputs["wgt"], f32)[l])))
              for l in range(L)]
    TR = np.zeros((L, R, P, 132), f32)
    for l in range(L):
        for r in range(R):
            a_src = nta[l, SRC_NT[r]] * wl_all[l]
            for hh in range(H):
                pri = rel_pri[l, r, hh] / SQRT_DK
                TR[l, r, hh * DK:(hh + 1) * DK, hh * DK:(hh + 1) * DK] = rel_att[l, r, hh] * pri
                TR[l, r, hh * DK:(hh + 1) * DK, 64 + hh] = afc[l, DK:] * a_src
                TR[l, r, 64 + hh * DK:64 + (hh + 1) * DK, 68 + hh * DK:68 + (hh + 1) * DK] = rel_msg[l, r, hh]

    Ps = np.zeros((len(pp["classes"]), P, P), f32)
    for ci, s in enumerate(pp["classes"]):
        cap = P // s
        Ps[ci, :, cap:] = 1e-30  # pad seg columns: tiny-but-finite denominators
        for j in range(cap):
            Ps[ci, j * s:(j + 1) * s, j] = 1.0

    afc1rep = np.zeros((L, P, D), f32)
    for l in range(L):
        afc1rep[l, :, :] = np.tile(afc[l, :DK], H)[None, :]

    scal = {
        "a_dst": [[float(nta1[l, DST_NT[r]]) * wl_all[l] for r in range(R)] for l in range(L)],
        "wl": wl_all,
        "alpha": [[float(1.0 / (1.0 + np.exp(-np.asarray(inputs["skipp"], f32)[l, t])))
                   for t in range(T)] for l in range(L)],
    }

    Wk = np.asarray(inputs["Wk"], f32); Wq = np.asarray(inputs["Wq"], f32)
    Wv = np.asarray(inputs["Wv"], f32)
    bk = np.asarray(inputs["bk"], f32); bq = np.asarray(inputs["bq"], f32)
    bv = np.asarray(inputs["bv"], f32)
    pWa = np.concatenate([Wk, Wq, Wv], axis=-1)[:L]
    pBa = np.concatenate([bk, bq, bv], axis=-1)[:L]
    pB_rep = np.broadcast_to(pBa[:, :, None, :], (L, T, P, 3 * D)).copy()

    shared = {
        "adW": np.asarray(inputs["adapt_W"], f32),
        "adB": np.asarray(inputs["adapt_b"], f32)[:, :, None],
        "pW": np.ascontiguousarray(pWa), "pB": pB_rep,
        "WaT": np.ascontiguousarray(np.asarray(inputs["Wa"], f32)[:L]),
        "baC": np.ascontiguousarray(np.asarray(inputs["ba"], f32)[:L][..., None]),
        "lnG": np.ascontiguousarray(np.asarray(inputs["ln_g"], f32)[:L][..., None]),
        "lnB": np.ascontiguousarray(np.asarray(inputs["ln_b"], f32)[:L][..., None]),
        "TRr": _bf16(TR), "Ps": _bf16(Ps),
        "afc1rep": afc1rep,
        "ident": np.eye(P, dtype=f32),
    }

    in_maps = []
    for c in range(NC_CORES):
        co = pp["cores"][c]
        hs = np.zeros((dn, D_IN), f32)
        ids = new2old[c * dn:(c + 1) * dn]
        rl = ids >= 0
        hs[rl] = h[ids[rl]]
        npi = (1.0 / pp["npres"][c]).reshape(nchunk, P).T
        m = dict(shared)
        m["h_shard"] = hs
        m["npres_inv"] = np.ascontiguousarray(npi)
        m["kv_idx"] = _wrap16(co["e_src"])
        m["q_idx"] = _wrap16(co["e_dstl"])
        m["sc_idx"] = _wrap16(co["seg_dstl"])
        in_maps.append(m)
    return in_maps, scal


# ---------------------------------------------------------------- entry
def kernel(**inputs):
    from concourse.bass_utils import run_bass_kernel_spmd

    L = int(os.environ.get("BRING_L", L_FULL))
    src = np.asarray(inputs["src"]); dst = np.asarray(inputs["dst"])
    etype = np.asarray(inputs["etype"]); ntype = np.asarray(inputs["ntype"])
    pp = _prep(src, dst, etype, ntype)
    in_maps, scal = _pack_inputs(pp, inputs, L)

    key = (L, pp["dn"], tuple(pp["nsub"].ravel().tolist()),
           tuple(pp["chunk_types"].tolist()),
           tuple(map(tuple, scal["a_dst"])), tuple(scal["wl"]),
           tuple(map(tuple, scal["alpha"])))
    if key not in _PROG_CACHE:
        _PROG_CACHE[key] = _build(pp, L, scal)
    nc = _PROG_CACHE[key]

    res = run_bass_kernel_spmd(
        nc, in_maps, list(range(NC_CORES)),
        trace=bool(int(os.environ.get("BASS_KERNEL_TRACE", "0"))))
    kernel.last_results = res
    full = np.concatenate([res.results[c]["x_out"] for c in range(NC_CORES)], axis=0)
    return full[pp["old2new"]].astype(np.float32)

